# revision 11
# baseline (speedup 1.0000x reference)
"""GQA attention (B=4,S=1024,D=2048,H=32,KVH=8,HD=64) + RoPE, tensor-parallel
over the 8 kv-head groups across 8 NeuronCores.

Per-core pipeline (all-transposed layouts, no on-device softmax-max pass):
  qT/kT/vT = W.T @ xT            (PE, bf16, D-contraction in 16 chunks of 128;
                                  all weights/x pre-swizzled on host so every
                                  DMA is a contiguous [128, N] block)
  RoPE: qrot = (q*cos) + A@(q*sin)    (rotate-half folded into a PE matmul A;
                                  k uses zero-padded rhs so the matmul stays
                                  in the standard 128x128 PE mode)
  attention inner loop per (qh-half, kb-pair):
    4 score matmuls on PE row groups (0,0)/(64,0)  [64-row tiled mode block]
    2 exp calls [128,1024] on ACT
    4 AV accumulation matmuls [standard mode block]
  (batching by kb-pair halves the PE array mode-switch drains)
  avn = avT * reciprocal(denom rows)   (DVE; odd head shifted to partitions
                                        64:128 by gpsimd SBUF-SBUF DMA)
  y_partial = avn_pairT.T @ Wo_pair    (PE; PSUM banks borrowed from proj
                                        pools; evacuation split ACT||DVE)
  y staged to [128, 2048] rows, stored by ring DMA on the sync queue;
  host sums the 8 partials in fp32.

PE is pre-warmed with dummy matmuls during the initial x DMA so the HAM
clock gate reaches 8/8 before the first projection; the ACT exp table is
preloaded the same way.
"""

import numpy as np
import ml_dtypes

import concourse.bass as bass
import concourse.mybir as mybir
import concourse.tile as tile
from concourse import bacc
from concourse import bass_utils

BF16 = mybir.dt.bfloat16
F32 = mybir.dt.float32
BF = ml_dtypes.bfloat16

B, S, D = 4, 1024, 2048
H, KVH, HD = 32, 8, 64
NREP = H // KVH          # 4 q heads per core
T = B * S                # 4096 tokens
NC = 8                   # cores
QD = NREP * HD           # 256 q dims per core
KC = D // 128            # 16 contraction chunks
TB = 512                 # proj token-block
AF = mybir.ActivationFunctionType

_CACHE = {}


def _build():
    if "nc" in _CACHE:
        return _CACHE["nc"]
    nc = bacc.Bacc("TRN2", target_bir_lowering=False)

    x_d = nc.dram_tensor("xp", (2 * B, 128, KC * TB), BF16, kind="ExternalInput")
    wq_d = nc.dram_tensor("wq", (128, KC * QD), BF16, kind="ExternalInput")
    wkv_d = nc.dram_tensor("wkv", (128, KC * 128), BF16, kind="ExternalInput")
    wo_d = nc.dram_tensor("wo", (128, 2 * D), BF16, kind="ExternalInput")
    cos_d = nc.dram_tensor("cos2", (128, S), F32, kind="ExternalInput")
    sin_d = nc.dram_tensor("sin2", (128, S), F32, kind="ExternalInput")
    arot_d = nc.dram_tensor("arot", (128, 128), BF16, kind="ExternalInput")
    y_d = nc.dram_tensor("y", (T, D), BF16, kind="ExternalOutput")

    with tile.TileContext(nc) as tc:
        with (
            tc.tile_pool(name="const", bufs=1) as cpool,
            tc.tile_pool(name="qk", bufs=4) as qkpool,
            tc.tile_pool(name="kv", bufs=2) as kvpool,
            tc.tile_pool(name="vt", bufs=2) as vtpool,
            tc.tile_pool(name="va", bufs=2) as vapool,
            tc.tile_pool(name="xin", bufs=4) as xpool,
            tc.tile_pool(name="rt", bufs=3) as rpool,
            tc.tile_pool(name="pb", bufs=4) as prpool,
            tc.tile_pool(name="nm", bufs=4) as npool,
            tc.tile_pool(name="an", bufs=4) as apool,
            tc.tile_pool(name="yo", bufs=3) as ypool,
            # 8 PSUM banks total: pacc 1 (proj accum / warmup / outproj),
            # pshv 1 (rope shift / outproj), pscr 4 (two [128,1024] score
            # tiles, depth-2 rotation over kb pairs), pav 2 (AV accum chains)
            tc.tile_pool(name="pacc", bufs=1, space="PSUM") as pacc,
            tc.tile_pool(name="pshv", bufs=1, space="PSUM") as pshv,
            tc.tile_pool(name="pscr", bufs=2, space="PSUM") as pscr,
            tc.tile_pool(name="pav", bufs=2, space="PSUM") as pav,
        ):
            # ---- constants + x, ordered by first use; every transfer is a
            # contiguous [128, N] block thanks to host-side swizzling.
            # sync and scalar are HW DGE queues; gpsimd is the software DGE.
            wq_sb = cpool.tile([128, KC * QD], BF16, tag="wq")
            wkv_sb = cpool.tile([128, KC * 128], BF16, tag="wkv")
            cos_sb = cpool.tile([128, S], F32, tag="cos")
            sin_sb = cpool.tile([128, S], F32, tag="sin")
            arot_sb = cpool.tile([128, 128], BF16, tag="arot")
            wo_sb = cpool.tile([128, 2 * D], BF16, tag="wo")
            scr_sb = cpool.tile([128, 128], BF16, tag="scr")

            nc.gpsimd.dma_start(out=arot_sb[:], in_=arot_d[:])
            nc.sync.dma_start(out=wq_sb[:, 0:4 * QD], in_=wq_d[:, 0:4 * QD])
            nc.scalar.dma_start(out=wkv_sb[:], in_=wkv_d[:])

            xts_all = {}

            def load_x(tb, queue, split=1):
                xts = xpool.tile([128, KC * TB], BF16, tag="xts", name=f"x{tb}")
                xts_all[tb] = xts
                n = KC * TB
                for i in range(split):
                    queue.dma_start(
                        out=xts[:, i * n // split:(i + 1) * n // split],
                        in_=x_d[tb][:, i * n // split:(i + 1) * n // split],
                    )

            load_x(0, nc.sync, split=4)
            nc.sync.dma_start(out=wq_sb[:, 4 * QD:], in_=wq_d[:, 4 * QD:])
            nc.scalar.dma_start(out=cos_sb[:, 0:TB], in_=cos_d[:, 0:TB])
            nc.scalar.dma_start(out=sin_sb[:, 0:TB], in_=sin_d[:, 0:TB])
            load_x(1, nc.sync, split=2)
            load_x(2, nc.scalar)
            nc.scalar.dma_start(out=cos_sb[:, TB:S], in_=cos_d[:, TB:S])
            nc.scalar.dma_start(out=sin_sb[:, TB:S], in_=sin_d[:, TB:S])
            load_x(3, nc.sync)

            # warm the PE (HAM clock gate) with dummy matmuls on arot while
            # the first x blocks are still in flight; also preload the ACT
            # exp table so the first real exp doesn't pay the table load.
            warm = pacc.tile([128, 128], F32, tag="acc", name="warmup")
            for i in range(40):
                nc.tensor.matmul(warm[:], arot_sb[:], arot_sb[:],
                                 start=True, stop=True)
            nc.scalar.activation(scr_sb[:], arot_sb[:], AF.Exp, scale=0.125)

            # persistent zero-padded rhs for the k rope shift: rows 64:128
            # stay zero so lhsT can be the full arot and the matmul runs in
            # the standard 128x128 mode (no PE mode-switch drain).
            ksin_sb = cpool.tile([128, TB], BF16, tag="ksin")
            nc.vector.memset(ksin_sb[64:128, :], 0.0)

            def load_wo():
                nc.gpsimd.dma_start(out=wo_sb[:], in_=wo_d[:])

            qrope = {}   # (b, pr) -> [128, S] bf16 (head 2pr rows 0:64, 2pr+1 rows 64:128)
            kT = {}      # b -> [128, S] bf16 (k_rope duplicated top/bottom)
            v_aug = {}   # b -> [128, 8*128] bf16 ([v | ones64] per key block)
            avn = {}     # (b, pr) -> [128, S] bf16 normalized AV pair

            def emit_proj(b):
                vT_t = vtpool.tile([128, S], BF16, tag="vT")
                kT_t = kvpool.tile([128, S], BF16, tag="kT")
                q_t = [qkpool.tile([128, S], BF16, tag="qr", name=f"qr{b}_{p}")
                       for p in range(2)]
                qrope[b] = q_t
                kT[b] = kT_t
                for half in range(2):
                    tb = 2 * b + half
                    scol = half * TB
                    xts = xts_all[tb]
                    css = cos_sb[:, scol:scol + TB]
                    sns = sin_sb[:, scol:scol + TB]
                    for st in range(3):
                        acc = pacc.tile([128, TB], F32, tag="acc", name=f"acc{tb}_{st}")
                        for c in range(KC):
                            if st < 2:
                                w = wq_sb[:, c * QD + st * 128:c * QD + (st + 1) * 128]
                            else:
                                w = wkv_sb[:, bass.ts(c, 128)]
                            nc.tensor.matmul(
                                acc[:], w, xts[:, bass.ts(c, TB)],
                                start=(c == 0), stop=(c == KC - 1),
                            )
                        if st < 2:
                            qsin = rpool.tile([128, TB], BF16, tag="qsin")
                            nc.vector.tensor_mul(qsin[:], acc[:], sns)
                            t1 = rpool.tile([128, TB], BF16, tag="t1")
                            nc.vector.tensor_mul(t1[:], acc[:], css)
                            sh = pshv.tile([128, TB], F32, tag="shv", name=f"sh{tb}_{st}")
                            nc.tensor.matmul(sh[:], arot_sb[:], qsin[:], start=True, stop=True)
                            nc.vector.tensor_add(q_t[st][:, scol:scol + TB], t1[:], sh[:])
                        else:
                            nc.vector.tensor_mul(ksin_sb[0:64, :], acc[0:64, :], sns[0:64])
                            t1k = rpool.tile([64, TB], BF16, tag="t1")
                            nc.vector.tensor_mul(t1k[:], acc[0:64, :], css[0:64])
                            sh = pshv.tile([128, TB], F32, tag="shv", name=f"sh{tb}_{st}")
                            nc.tensor.matmul(sh[:], arot_sb[:], ksin_sb[:],
                                             start=True, stop=True)
                            nc.vector.tensor_add(kT_t[0:64, scol:scol + TB], t1k[:], sh[0:64, :])
                            nc.gpsimd.dma_start(
                                out=kT_t[64:128, scol:scol + TB],
                                in_=kT_t[0:64, scol:scol + TB],
                            )
                            nc.vector.tensor_copy(vT_t[64:128, scol:scol + TB], acc[64:128, :])
                    # prefetch x for the next-but-one proj while this one runs
                    nx = tb + 4
                    if 4 <= nx < 2 * B:
                        load_x(nx, nc.sync)
                # v natural (+ ones block) per key block of 128, transposed
                # off the PE via the XBAR DMA transpose on the sync queue
                va_t = vapool.tile([128, 8 * 128], BF16, tag="vaug")
                v_aug[b] = va_t
                vav = va_t[:].rearrange("p (k c) -> p k c", k=8)
                nc.vector.memset(vav[:, :, 64:128], 1.0)
                for kb in range(8):
                    nc.sync.dma_start_transpose(
                        out=vav[:, kb, 0:64],
                        in_=vT_t[64:128, kb * 128:(kb + 1) * 128],
                    )

            def emit_attn_qh(b, pr, qh):
                # one qh-half (512 q tokens), kb in pairs: a 4-matmul score
                # block on PE row groups (0,0)/(64,0), two exps, then a
                # 4-matmul AV block in standard mode. Batching by kb-pair
                # halves the PE mode-switch drains and lets the row-group
                # pairs run concurrently.
                q_t = qrope[b][pr]
                col = qh * 512
                if qh == 0:
                    avn[(b, pr)] = apool.tile([128, S], BF16, tag="avn",
                                              name=f"avn{b}_{pr}")
                avn_t = avn[(b, pr)]
                avA = pav.tile([128, 512], F32, tag="av", name=f"avA{b}{pr}{qh}")
                avB = pav.tile([128, 512], F32, tag="av", name=f"avB{b}{pr}{qh}")
                for kp in range(4):
                    sAB = []
                    for j in range(2):
                        kb = 2 * kp + j
                        s_t = pscr.tile([128, 1024], F32, tag="scr",
                                        name=f"s{b}{pr}{qh}{kb}")
                        sAB.append(s_t)
                        nc.tensor.matmul(
                            s_t[:, 0:512],
                            kT[b][0:64, kb * 128:(kb + 1) * 128],
                            q_t[0:64, col:col + 512],
                            start=True, stop=True, tile_position=(0, 0),
                        )
                        nc.tensor.matmul(
                            s_t[:, 512:1024],
                            kT[b][64:128, kb * 128:(kb + 1) * 128],
                            q_t[64:128, col:col + 512],
                            start=True, stop=True, tile_position=(64, 0),
                        )
                    pAB = []
                    for j in range(2):
                        p_t = prpool.tile([128, 1024], BF16, tag="probs",
                                          name=f"p{b}{pr}{qh}{2 * kp + j}")
                        pAB.append(p_t)
                        nc.scalar.activation(p_t[:], sAB[j][:], AF.Exp, scale=0.125)
                    for j in range(2):
                        kb = 2 * kp + j
                        nc.tensor.matmul(
                            avA[:], v_aug[b][:, kb * 128:(kb + 1) * 128],
                            pAB[j][:, 0:512], start=(kb == 0), stop=(kb == 7),
                        )
                        nc.tensor.matmul(
                            avB[:], v_aug[b][:, kb * 128:(kb + 1) * 128],
                            pAB[j][:, 512:1024], start=(kb == 0), stop=(kb == 7),
                        )
                for h, avp in ((0, avA), (1, avB)):
                    den = npool.tile([64, 512], F32, tag="den")
                    nc.vector.tensor_copy(den[:], avp[64:128, :])
                    rbc = npool.tile([64, 512], F32, tag="rbc")
                    nc.vector.reciprocal_approx_fast(rbc[:], den[:])
                    if h == 0:
                        nc.vector.tensor_mul(
                            avn_t[0:64, col:col + 512], avp[0:64, :], rbc[:]
                        )
                    else:
                        aodd = npool.tile([64, 512], BF16, tag="aodd")
                        nc.vector.tensor_mul(aodd[:], avp[0:64, :], rbc[:])
                        nc.gpsimd.dma_start(
                            out=avn_t[64:128, col:col + 512], in_=aodd[:]
                        )

            def emit_attn(b, pr):
                emit_attn_qh(b, pr, 0)
                emit_attn_qh(b, pr, 1)

            def emit_outproj(b, trange):
                # yp tiles rotate over the pacc/pshv banks (proj is done or
                # far ahead); evacuation is split ACT || DVE halves so the
                # bank frees in ~390ns and the depth-2 rotation never stalls
                # the PE. y rows staged to SBUF, one ring DMA per [128, 2048].
                for t in trange:
                    ys = ypool.tile([128, D], BF16, tag="ys", name=f"ys{b}_{t}")
                    for nb in range(4):
                        pool = pacc if nb % 2 == 0 else pshv
                        yp = pool.tile([128, 512], F32, tag="acc" if nb % 2 == 0 else "shv",
                                       name=f"yp{b}{t}{nb}")
                        for p in range(2):
                            nc.tensor.matmul(
                                yp[:],
                                avn[(b, p)][:, t * 128:(t + 1) * 128],
                                wo_sb[:, p * D + nb * 512:p * D + (nb + 1) * 512],
                                start=(p == 0), stop=(p == 1),
                            )
                        c0 = nb * 512
                        nc.scalar.copy(ys[:, c0:c0 + 256], yp[:, 0:256])
                        nc.vector.tensor_copy(ys[:, c0 + 256:c0 + 512], yp[:, 256:512])
                    nc.sync.dma_start(
                        out=y_d[b * S + t * 128:b * S + (t + 1) * 128, :],
                        in_=ys[:],
                    )

            # Interleave proj(b+2) and outproj(b-1) with attention(b) so the
            # PE always has ready matmul work while ACT runs exp; the b=3
            # outproj is split so its first half overlaps attn(3,1).
            emit_proj(0)
            emit_proj(1)
            load_wo()
            emit_attn(0, 0)
            emit_attn(0, 1)
            emit_proj(2)
            emit_attn(1, 0)
            emit_proj(3)
            emit_attn(1, 1)
            emit_outproj(0, range(8))
            emit_attn(2, 0)
            emit_outproj(1, range(8))
            emit_attn(2, 1)
            emit_outproj(2, range(8))
            emit_attn(3, 0)
            emit_attn_qh(3, 1, 0)
            emit_outproj(3, range(4))
            emit_attn_qh(3, 1, 1)
            emit_outproj(3, range(4, 8))

    nc.compile()
    _CACHE["nc"] = nc
    return nc


def _host_prep(x, cos, sin, Wq, Wk, Wv, Wo):
    x = np.asarray(x, np.float32)
    # xp[tb, p, c*TB+n] = x[tb*TB+n, c*128+p] -> contiguous [128, 8192] loads
    xp = np.ascontiguousarray(
        x.reshape(2 * B, TB, KC, 128).transpose(0, 3, 2, 1).reshape(2 * B, 128, KC * TB)
    ).astype(BF)
    cosT = np.asarray(cos, np.float32).T
    sinT = np.asarray(sin, np.float32).T
    cos2 = np.ascontiguousarray(np.tile(cosT, (2, 1)))          # (128, S) f32
    sin2 = np.ascontiguousarray(np.tile(sinT, (2, 1)))
    # lhsT for qshiftT = A @ qT  ->  arot = A.T (block-diag x2 over heads)
    A = np.zeros((HD, HD), np.float32)
    for d in range(32):
        A[d, d + 32] = -1.0
        A[32 + d, d] = 1.0
    arot = np.kron(np.eye(2, dtype=np.float32), A.T).astype(BF)  # (128,128)

    Wq = np.asarray(Wq, np.float32)
    Wk = np.asarray(Wk, np.float32)
    Wv = np.asarray(Wv, np.float32)
    Wo = np.asarray(Wo, np.float32)
    in_maps = []
    for g in range(NC):
        wq_g = Wq[:, g * QD:(g + 1) * QD]
        wqp = np.ascontiguousarray(
            wq_g.reshape(KC, 128, QD).transpose(1, 0, 2).reshape(128, KC * QD)
        ).astype(BF)
        wkv_g = np.concatenate(
            [Wk[:, g * HD:(g + 1) * HD], Wv[:, g * HD:(g + 1) * HD]], axis=1
        )
        wkvp = np.ascontiguousarray(
            wkv_g.reshape(KC, 128, 128).transpose(1, 0, 2).reshape(128, KC * 128)
        ).astype(BF)
        wo_g = Wo[g * QD:(g + 1) * QD, :]
        wop = np.ascontiguousarray(
            wo_g.reshape(2, 128, D).transpose(1, 0, 2).reshape(128, 2 * D)
        ).astype(BF)
        in_maps.append({
            "xp": xp, "wq": wqp, "wkv": wkvp, "wo": wop,
            "cos2": cos2, "sin2": sin2, "arot": arot,
        })
    return in_maps


def kernel(x, cos, sin, Wq, Wk, Wv, Wo):
    nc = _build()
    in_maps = _host_prep(x, cos, sin, Wq, Wk, Wv, Wo)
    res = bass_utils.run_bass_kernel_spmd(
        nc, in_maps, core_ids=list(range(NC)), trace=False,
    )
    y = np.zeros((T, D), np.float32)
    for r in res.results:
        y += np.asarray(r["y"], np.float32)
    return y.reshape(B, S, D)
